# revision 13
# baseline (speedup 1.0000x reference)
"""BiAttention similarity kernel for Trainium2, 8-core data-parallel over batch.

Computes, per batch b:
    s0 = c @ c_weight                  # [L, 1]
    s1 = (c @ q_weight)^T              # [1, L]
    s2 = (c * cq_weight) @ q^T         # [L, L]
    s  = s0 + s1 + s2 + bias           # [L, L]

Shapes (hardcoded): B=8, L=2048, D=256, fp32 in/out.

Distribution: data-parallel over batch, one batch per core. Host hands each
core its shard d-major (transposed) fp16 plus a packed per-partition weight
tile; the device returns s in fp16 (quantization ~5e-4 rel, well under
tolerance) and the host upcasts to fp32 — halving the dominant HBM write.

Device dataflow per core:
  - warmup: a few dummy matmuls while inputs stream in, so the PE HAM clock
    gate reaches 8/8 before real work starts.
  - inputs: ct halves first (one per HWDGE ring), qt halves second — each
    ring drains FIFO, so c^T lands early for the s1 phase.
  - s1 broadcast computed directly: lhsT = qwb (q_weight replicated across
    all 128 output partitions) gives s1b[p, j] = sum_d qw[d] c^T[d, j] in
    one matmul pass; +bias folds into the fp16 PSUM->SBUF copies
    (per-partition AP bias, bias replicated down a wconsts column).
  - qmod = q^T * cq_weight on DVE, A columns first.
  - main loop over 16 row chunks: PE fills one [128,512] A tile + one
    [128,1536] B tile per chunk (2 K-chunks of 128), plus a tiny N=1 matmul
    per K-chunk computing s0 into a persistent PSUM column (~26 ns of issue
    each, rides the same LDWEIGHTS; bank pre-cleared once so these run
    start=False and don't fake-conflict with the s0 column reads).
  - drains: DVE scalar_tensor_tensor fuses (psumA + s0) + s1b -> fp16;
    ACT folds the s0 add into its fp32->fp16 copy of B (Identity + AP
    bias); DVE adds s1b over B in an all-fp16 2x-mode tensor_tensor.
  - one contiguous 512 KiB output DMA per chunk on the sync ring; the last
    chunk is drained B-first with split DMAs to shorten the tail.
"""

import numpy as np
from contextlib import ExitStack

import concourse.bass as bass
import concourse.tile as tile
from concourse import bacc, mybir
from concourse.bass_utils import run_bass_kernel_spmd

F32 = mybir.dt.float32
F16 = mybir.dt.float16
ADD = mybir.AluOpType.add

B = 8
L = 2048
D = 256
NK = D // 128          # 2 contraction chunks of 128
NI = L // 128          # 16 row chunks
ASPLIT = 512           # A = [0:512] (DVE fused drain), B = [512:2048] (ACT)
N_WARMUP = 6          # dummy matmuls to warm the PE clock gate

TRACE = False
LAST_RESULTS = None

_NC_CACHE = None


def build_body(ctx: ExitStack, tc: tile.TileContext, aps: dict):
    nc = tc.nc
    ct_d, qt_d, w_d, s_d = aps["ct"], aps["qt"], aps["wconsts"], aps["s"]
    Copy = mybir.ActivationFunctionType.Copy

    consts = ctx.enter_context(tc.tile_pool(name="consts", bufs=1))
    psA = ctx.enter_context(tc.tile_pool(name="psA", bufs=1, space="PSUM"))
    psB = ctx.enter_context(tc.tile_pool(name="psB", bufs=2, space="PSUM"))
    ps0 = ctx.enter_context(tc.tile_pool(name="ps0", bufs=1, space="PSUM"))
    outp = ctx.enter_context(tc.tile_pool(name="outp", bufs=6))

    # ---- constants -------------------------------------------------------
    # host-packed [128, 7] fp32: cols 0-1 cw(k0,k1), 2-3 qw, 4-5 cqw,
    # 6 bias replicated down all partitions
    # (memsets emitted first so the PE warmup isn't gated on the DMA)
    ones16 = consts.tile([1, 128], F16)
    nc.gpsimd.memset(ones16[0:1, :], 1.0)
    dummy16 = consts.tile([1, 512], F16)
    nc.gpsimd.memset(dummy16[0:1, :], 0.0)
    qwb = [consts.tile([128, 128], F16, tag=f"qwb{k}", name=f"qwb{k}")
           for k in range(NK)]
    for k in range(NK):
        nc.gpsimd.memset(qwb[k][:, :], 1.0)
    wc = consts.tile([128, 7], F32)
    nc.gpsimd.dma_start(wc[:], w_d)
    cw16 = consts.tile([128, NK], F16)
    nc.vector.tensor_copy(cw16[:], wc[:, 0:2])
    # qwb_k[d, m] = q_weight[d] for all m: broadcast via per-partition scale
    for k in range(NK):
        nc.vector.tensor_scalar_mul(qwb[k][:, :], qwb[k][:, :],
                                    wc[:, 2 + k:3 + k])

    # ---- PE warmup while inputs stream ----------------------------------
    warm = psA.tile([128, ASPLIT], F32, tag="A", name="warm")
    for w in range(N_WARMUP):
        nc.tensor.matmul(warm[:], ones16[0:1, :], dummy16[0:1, :],
                         start=True, stop=True)

    # ---- inputs: ct halves first on both rings, then qt ------------------
    cT = [consts.tile([128, L], F16, tag=f"cT{k}", name=f"cT{k}")
          for k in range(NK)]
    qT = [consts.tile([128, L], F16, tag=f"qT{k}", name=f"qT{k}")
          for k in range(NK)]
    nc.sync.dma_start(cT[0][:, :], ct_d[0:128, :])
    nc.scalar.dma_start(cT[1][:, :], ct_d[128:256, :])
    nc.sync.dma_start(qT[0][:, :], qt_d[0:128, :])
    nc.scalar.dma_start(qT[1][:, :], qt_d[128:256, :])

    # ---- s1 broadcast, directly: s1b[p, j] = sum_d qw[d] * cT[d, j] ------
    # (lhsT = qwb so every output partition gets the same s1 row; no [1,L]
    # row stage, no separate broadcast matmul)
    s1b16 = consts.tile([128, L], F16)
    s1ps = [psB.tile([128, 1024], F32, tag="B", name=f"s1ps{h}",
                     padded_shape=[128, 1536])
            for h in range(2)]
    for k in range(NK):
        for jj in range(4):
            nc.tensor.matmul(
                s1ps[jj // 2][:, (jj % 2) * 512:(jj % 2) * 512 + 512],
                qwb[k][:, :], cT[k][:, jj * 512:(jj + 1) * 512],
                start=(k == 0), stop=(k == NK - 1))
    # fp16 copies with the bias add folded in (per-partition AP bias):
    # DVE makes the A-side cols, ACT the rest
    nc.vector.tensor_scalar_add(s1b16[:, 0:ASPLIT], s1ps[0][:, 0:ASPLIT],
                                wc[:, 6:7])
    nc.scalar.add(s1b16[:, ASPLIT:1024], s1ps[0][:, ASPLIT:1024], wc[:, 6:7])
    nc.scalar.add(s1b16[:, 1024:L], s1ps[1][:, 0:1024], wc[:, 6:7])

    # qmod = qT * cq_weight (per-partition scalar), on DVE, A columns first
    # so the first chunk's A matmuls unblock as soon as possible
    for k in range(NK):
        nc.vector.tensor_scalar_mul(qT[k][:, 0:ASPLIT], qT[k][:, 0:ASPLIT],
                                    wc[:, 4 + k:5 + k])
    for k in range(NK):
        nc.vector.tensor_scalar_mul(qT[k][:, ASPLIT:L], qT[k][:, ASPLIT:L],
                                    wc[:, 4 + k:5 + k])

    # ---- main loop: 16 row chunks ----------------------------------------
    s0c_ps = ps0.tile([128, NI], F32, tag="s0c", name="s0c_ps")
    s0_sb = consts.tile([128, NI], F32)
    # one start=True matmul clears the bank's has_written bits and zeroes
    # the s0 columns; the per-chunk matmuls then run start=False so they
    # only touch their own column (no whole-bank clear -> no false WAR
    # against the per-chunk s0 copies)
    nc.tensor.matmul(s0c_ps[:, :], ones16[0:1, :], dummy16[0:1, 0:NI],
                     start=True, stop=True)

    for i in range(NI):
        isl = slice(i * 128, (i + 1) * 128)
        last_chunk = (i == NI - 1)
        out_sb = outp.tile([128, L], F16, tag="out", name="out_sb")
        pa = psA.tile([128, ASPLIT], F32, tag="A", name="pa")
        pb = psB.tile([128, L - ASPLIT], F32, tag="B", name="pb")
        for k in range(NK):
            first, last = (k == 0), (k == NK - 1)
            # tiny s0 matmul rides the same LDWEIGHTS as the main matmuls;
            # start=False always (bank pre-cleared above): first write
            # overwrites (bit clear), second accumulates (bit set)
            nc.tensor.matmul(s0c_ps[:, i:i + 1], cT[k][:, isl],
                             cw16[:, k:k + 1], start=False, stop=last,
                             skip_group_check=True)
            mms = [(pb, jj) for jj in range(3)] + [(pa, None)]
            if last_chunk:
                mms = mms  # B tiles first, A last: shortens the drain tail
            else:
                mms = [(pa, None)] + [(pb, jj) for jj in range(3)]
            for ps, jj in mms:
                if jj is None:
                    nc.tensor.matmul(pa[:], cT[k][:, isl],
                                     qT[k][:, 0:ASPLIT],
                                     start=first, stop=last)
                else:
                    nc.tensor.matmul(pb[:, jj * 512:(jj + 1) * 512],
                                     cT[k][:, isl],
                                     qT[k][:, ASPLIT + jj * 512:
                                            ASPLIT + (jj + 1) * 512],
                                     start=first, stop=last)
        # s0 column for this chunk -> SBUF (tiny DVE copy; ACT bias needs SBUF)
        nc.vector.tensor_copy(s0_sb[:, i:i + 1], s0c_ps[:, i:i + 1])

        def drain_a():
            # A: one fused DVE op
            nc.vector.scalar_tensor_tensor(
                out_sb[:, 0:ASPLIT], pa[:], s0_sb[:, i:i + 1],
                s1b16[:, 0:ASPLIT], ADD, ADD)

        def drain_b():
            # B: ACT folds the s0 add into the fp32->fp16 copy, DVE adds
            # s1b in an all-fp16 2x-mode pass
            nc.scalar.add(out_sb[:, ASPLIT:L], pb[:], s0_sb[:, i:i + 1])
            nc.vector.tensor_add(out_sb[:, ASPLIT:L], out_sb[:, ASPLIT:L],
                                 s1b16[:, ASPLIT:L])

        if last_chunk:
            # B filled first: drain + ship it while A finishes
            drain_b()
            nc.sync.dma_start(s_d[isl, ASPLIT:L], out_sb[:, ASPLIT:L])
            drain_a()
            nc.sync.dma_start(s_d[isl, 0:ASPLIT], out_sb[:, 0:ASPLIT])
        else:
            # stt32 first on the DVE FIFO so the single-buffered A tile
            # frees before the next chunk's fill needs it
            drain_a()
            drain_b()
            nc.sync.dma_start(s_d[isl, :], out_sb[:, :])


def build_nc():
    nc = bacc.Bacc("TRN2", target_bir_lowering=False, debug=False)
    aps = {
        "ct": nc.dram_tensor("ct", [D, L], F16, kind="ExternalInput").ap(),
        "qt": nc.dram_tensor("qt", [D, L], F16, kind="ExternalInput").ap(),
        "wconsts": nc.dram_tensor("wconsts", [128, 7], F32,
                                  kind="ExternalInput").ap(),
        "s": nc.dram_tensor("s", [L, L], F16, kind="ExternalOutput").ap(),
    }
    with tile.TileContext(nc) as tc:
        with ExitStack() as ctx:
            build_body(ctx, tc, aps)
    nc.compile()
    return nc


def get_nc():
    global _NC_CACHE
    if _NC_CACHE is None:
        _NC_CACHE = build_nc()
    return _NC_CACHE


def kernel(c, q, c_weight, q_weight, cq_weight, bias):
    global LAST_RESULTS
    nc = get_nc()
    c = np.asarray(c, dtype=np.float32)
    q = np.asarray(q, dtype=np.float32)
    cw = np.asarray(c_weight, dtype=np.float32).reshape(2, 128).T  # [128, 2]
    qw = np.asarray(q_weight, dtype=np.float32).reshape(2, 128).T
    cqw = np.asarray(cq_weight, dtype=np.float32).reshape(2, 128).T
    bias = np.asarray(bias, dtype=np.float32)
    wconsts = np.zeros((128, 7), dtype=np.float32)
    wconsts[:, 0:2] = cw
    wconsts[:, 2:4] = qw
    wconsts[:, 4:6] = cqw
    wconsts[:, 6] = bias[0]
    in_maps = [
        {
            "ct": np.ascontiguousarray(c[b].T).astype(np.float16),
            "qt": np.ascontiguousarray(q[b].T).astype(np.float16),
            "wconsts": wconsts,
        }
        for b in range(B)
    ]
    res = run_bass_kernel_spmd(nc, in_maps, core_ids=list(range(B)), trace=TRACE)
    LAST_RESULTS = res
    return np.stack([res.results[b]["s"].astype(np.float32) for b in range(B)],
                    axis=0)


# revision 16
# speedup vs baseline: 1.1491x; 1.1491x over previous
"""BiAttention similarity kernel for Trainium2, 8-core data-parallel over batch.

Computes, per batch b:
    s0 = c @ c_weight                  # [L, 1]
    s1 = (c @ q_weight)^T              # [1, L]
    s2 = (c * cq_weight) @ q^T         # [L, L]
    s  = s0 + s1 + s2 + bias           # [L, L]

Shapes (hardcoded): B=8, L=2048, D=256, fp32 in/out.

Distribution: data-parallel over batch, one batch per core. Host hands each
core its shard d-major (transposed) fp16 plus a packed per-partition weight
tile; the device returns s in fp16 (quantization ~5e-4 rel, well under
tolerance) and the host upcasts to fp32 — halving the dominant HBM write.

Device dataflow per core:
  - warmup: a few dummy matmuls while inputs stream in, so the PE HAM clock
    gate reaches 8/8 before real work starts.
  - inputs: ct halves first (one per HWDGE ring), qt halves second — each
    ring drains FIFO, so c^T lands early for the s1 phase.
  - s1 broadcast computed directly: lhsT = qwb (q_weight replicated across
    all 128 output partitions) gives s1b[p, j] = sum_d qw[d] c^T[d, j] in
    one matmul pass; +bias folds into the fp16 PSUM->SBUF copies
    (per-partition AP bias, bias replicated down a wconsts column).
  - qmod = q^T * cq_weight on DVE, A columns first.
  - main loop over 16 row chunks: PE fills one [128,512] A tile + one
    [128,1536] B tile per chunk (2 K-chunks of 128), plus a tiny N=1 matmul
    per K-chunk computing s0 into a persistent PSUM column (~26 ns of issue
    each, rides the same LDWEIGHTS; bank pre-cleared once so these run
    start=False and don't fake-conflict with the s0 column reads).
  - drains: DVE scalar_tensor_tensor fuses (psumA + s0) + s1b -> fp16;
    ACT folds the s0 add into its fp32->fp16 copy of B (Identity + AP
    bias); DVE adds s1b over B in an all-fp16 2x-mode tensor_tensor.
  - one contiguous 512 KiB output DMA per chunk on the sync ring; the last
    chunk is drained B-first with split DMAs to shorten the tail.
"""

import numpy as np
from contextlib import ExitStack

import concourse.bass as bass
import concourse.tile as tile
from concourse import bacc, mybir
from concourse.bass_utils import run_bass_kernel_spmd

F32 = mybir.dt.float32
F16 = mybir.dt.float16
ADD = mybir.AluOpType.add

B = 8
L = 2048
D = 256
NK = D // 128          # 2 contraction chunks of 128
NI = L // 128          # 16 row chunks
ASPLIT = 512           # A = [0:512] (DVE fused drain), B = [512:2048] (ACT)
N_WARMUP = 8          # dummy matmuls to warm the PE clock gate

TRACE = False
LAST_RESULTS = None

_NC_CACHE = None


def build_body(ctx: ExitStack, tc: tile.TileContext, aps: dict):
    nc = tc.nc
    ct_d, qt_d, w_d, s_d = aps["ct"], aps["qt"], aps["wconsts"], aps["s"]
    Copy = mybir.ActivationFunctionType.Copy

    consts = ctx.enter_context(tc.tile_pool(name="consts", bufs=1))
    psA = ctx.enter_context(tc.tile_pool(name="psA", bufs=1, space="PSUM"))
    psB = ctx.enter_context(tc.tile_pool(name="psB", bufs=2, space="PSUM"))
    ps0 = ctx.enter_context(tc.tile_pool(name="ps0", bufs=1, space="PSUM"))
    outp = ctx.enter_context(tc.tile_pool(name="outp", bufs=6))

    # ---- constants -------------------------------------------------------
    # host-packed [128, 7] fp32: cols 0-1 cw(k0,k1), 2-3 qw, 4-5 cqw,
    # 6 bias replicated down all partitions
    # (memsets emitted first so the PE warmup isn't gated on the DMA)
    ones16 = consts.tile([1, 128], F16)
    nc.gpsimd.memset(ones16[0:1, :], 1.0)
    dummy16 = consts.tile([1, 512], F16)
    nc.gpsimd.memset(dummy16[0:1, :], 0.0)
    qwb = [consts.tile([128, 128], F16, tag=f"qwb{k}", name=f"qwb{k}")
           for k in range(NK)]
    for k in range(NK):
        nc.gpsimd.memset(qwb[k][:, :], 1.0)
    # wc loads FIRST on the sync HWDGE ring: per-ring FIFO drains its tiny
    # packets before the big ct/qt transfers, so the weights land in ~1 us
    # instead of losing the packet round-robin and finishing last
    wc = consts.tile([128, 7], F32)
    nc.sync.dma_start(wc[:], w_d)
    cw16 = consts.tile([128, NK], F16)
    nc.vector.tensor_copy(cw16[:], wc[:, 0:2])
    # qwb_k[d, m] = q_weight[d] for all m: broadcast via per-partition scale
    for k in range(NK):
        nc.vector.tensor_scalar_mul(qwb[k][:, :], qwb[k][:, :],
                                    wc[:, 2 + k:3 + k])

    # ---- PE warmup while inputs stream ----------------------------------
    warm = psA.tile([128, ASPLIT], F32, tag="A", name="warm")
    for w in range(N_WARMUP):
        nc.tensor.matmul(warm[:], ones16[0:1, :], dummy16[0:1, :],
                         start=True, stop=True)

    # ---- inputs: ct halves first on both rings, then qt ------------------
    cT = [consts.tile([128, L], F16, tag=f"cT{k}", name=f"cT{k}")
          for k in range(NK)]
    qT = [consts.tile([128, L], F16, tag=f"qT{k}", name=f"qT{k}")
          for k in range(NK)]
    nc.sync.dma_start(cT[0][:, :], ct_d[0:128, :])
    nc.scalar.dma_start(cT[1][:, :], ct_d[128:256, :])
    nc.sync.dma_start(qT[0][:, :], qt_d[0:128, :])
    nc.scalar.dma_start(qT[1][:, :], qt_d[128:256, :])

    # ---- s1 broadcast, directly: s1b[p, j] = sum_d qw[d] * cT[d, j] ------
    # (lhsT = qwb so every output partition gets the same s1 row; no [1,L]
    # row stage, no separate broadcast matmul)
    s1b16 = consts.tile([128, L], F16)
    s1ps = [psB.tile([128, 1024], F32, tag="B", name=f"s1ps{h}",
                     padded_shape=[128, 1536])
            for h in range(2)]
    for k in range(NK):
        for jj in range(4):
            nc.tensor.matmul(
                s1ps[jj // 2][:, (jj % 2) * 512:(jj % 2) * 512 + 512],
                qwb[k][:, :], cT[k][:, jj * 512:(jj + 1) * 512],
                start=(k == 0), stop=(k == NK - 1))
    # qmod = qT * cq_weight (per-partition scalar), on DVE: A columns first
    # so the first chunk's A matmuls unblock as soon as qt lands, then the
    # A-side s1b copy, then the B columns
    for k in range(NK):
        nc.vector.tensor_scalar_mul(qT[k][:, 0:ASPLIT], qT[k][:, 0:ASPLIT],
                                    wc[:, 4 + k:5 + k])
    # fp16 copies with the bias add folded in (per-partition AP bias):
    # DVE makes the A-side cols, ACT the rest
    nc.vector.tensor_scalar_add(s1b16[:, 0:ASPLIT], s1ps[0][:, 0:ASPLIT],
                                wc[:, 6:7])
    nc.scalar.add(s1b16[:, ASPLIT:1024], s1ps[0][:, ASPLIT:1024], wc[:, 6:7])
    nc.scalar.add(s1b16[:, 1024:L], s1ps[1][:, 0:1024], wc[:, 6:7])
    for k in range(NK):
        nc.vector.tensor_scalar_mul(qT[k][:, ASPLIT:L], qT[k][:, ASPLIT:L],
                                    wc[:, 4 + k:5 + k])

    # ---- main loop: 16 row chunks ----------------------------------------
    s0c_ps = ps0.tile([128, NI], F32, tag="s0c", name="s0c_ps")
    s0_sb = consts.tile([128, NI], F32)
    # one start=True matmul clears the bank's has_written bits and zeroes
    # the s0 columns; the per-chunk matmuls then run start=False so they
    # only touch their own column (no whole-bank clear -> no false WAR
    # against the per-chunk s0 copies)
    nc.tensor.matmul(s0c_ps[:, :], ones16[0:1, :], dummy16[0:1, 0:NI],
                     start=True, stop=True)

    for i in range(NI):
        isl = slice(i * 128, (i + 1) * 128)
        last_chunk = (i == NI - 1)
        out_sb = outp.tile([128, L], F16, tag="out", name="out_sb")
        pa = psA.tile([128, ASPLIT], F32, tag="A", name="pa")
        pb = psB.tile([128, L - ASPLIT], F32, tag="B", name="pb")
        for k in range(NK):
            first, last = (k == 0), (k == NK - 1)
            # tiny s0 matmul rides the same LDWEIGHTS as the main matmuls;
            # start=False always (bank pre-cleared above): first write
            # overwrites (bit clear), second accumulates (bit set)
            nc.tensor.matmul(s0c_ps[:, i:i + 1], cT[k][:, isl],
                             cw16[:, k:k + 1], start=False, stop=last,
                             skip_group_check=True)
            mms = [(pb, jj) for jj in range(3)] + [(pa, None)]
            if last_chunk:
                mms = mms  # B tiles first, A last: shortens the drain tail
            else:
                mms = [(pa, None)] + [(pb, jj) for jj in range(3)]
            for ps, jj in mms:
                if jj is None:
                    nc.tensor.matmul(pa[:], cT[k][:, isl],
                                     qT[k][:, 0:ASPLIT],
                                     start=first, stop=last)
                else:
                    nc.tensor.matmul(pb[:, jj * 512:(jj + 1) * 512],
                                     cT[k][:, isl],
                                     qT[k][:, ASPLIT + jj * 512:
                                            ASPLIT + (jj + 1) * 512],
                                     start=first, stop=last)
        # s0 column for this chunk -> SBUF (tiny DVE copy; ACT bias needs SBUF)
        nc.vector.tensor_copy(s0_sb[:, i:i + 1], s0c_ps[:, i:i + 1])

        def drain_a():
            # A: one fused DVE op
            nc.vector.scalar_tensor_tensor(
                out_sb[:, 0:ASPLIT], pa[:], s0_sb[:, i:i + 1],
                s1b16[:, 0:ASPLIT], ADD, ADD)

        def drain_b():
            # B: ACT folds the s0 add into the fp32->fp16 copy, DVE adds
            # s1b in an all-fp16 2x-mode pass
            nc.scalar.add(out_sb[:, ASPLIT:L], pb[:], s0_sb[:, i:i + 1])
            nc.vector.tensor_add(out_sb[:, ASPLIT:L], out_sb[:, ASPLIT:L],
                                 s1b16[:, ASPLIT:L])

        if last_chunk:
            # B filled first: drain + ship it while A finishes
            drain_b()
            nc.sync.dma_start(s_d[isl, ASPLIT:L], out_sb[:, ASPLIT:L])
            drain_a()
            nc.sync.dma_start(s_d[isl, 0:ASPLIT], out_sb[:, 0:ASPLIT])
        else:
            # stt32 first on the DVE FIFO so the single-buffered A tile
            # frees before the next chunk's fill needs it
            drain_a()
            drain_b()
            nc.sync.dma_start(s_d[isl, :], out_sb[:, :])


def build_nc():
    nc = bacc.Bacc("TRN2", target_bir_lowering=False, debug=False)
    aps = {
        "ct": nc.dram_tensor("ct", [D, L], F16, kind="ExternalInput").ap(),
        "qt": nc.dram_tensor("qt", [D, L], F16, kind="ExternalInput").ap(),
        "wconsts": nc.dram_tensor("wconsts", [128, 7], F32,
                                  kind="ExternalInput").ap(),
        "s": nc.dram_tensor("s", [L, L], F16, kind="ExternalOutput").ap(),
    }
    with tile.TileContext(nc) as tc:
        with ExitStack() as ctx:
            build_body(ctx, tc, aps)
    nc.compile()
    return nc


def get_nc():
    global _NC_CACHE
    if _NC_CACHE is None:
        _NC_CACHE = build_nc()
    return _NC_CACHE


def kernel(c, q, c_weight, q_weight, cq_weight, bias):
    global LAST_RESULTS
    nc = get_nc()
    c = np.asarray(c, dtype=np.float32)
    q = np.asarray(q, dtype=np.float32)
    cw = np.asarray(c_weight, dtype=np.float32).reshape(2, 128).T  # [128, 2]
    qw = np.asarray(q_weight, dtype=np.float32).reshape(2, 128).T
    cqw = np.asarray(cq_weight, dtype=np.float32).reshape(2, 128).T
    bias = np.asarray(bias, dtype=np.float32)
    wconsts = np.zeros((128, 7), dtype=np.float32)
    wconsts[:, 0:2] = cw
    wconsts[:, 2:4] = qw
    wconsts[:, 4:6] = cqw
    wconsts[:, 6] = bias[0]
    in_maps = [
        {
            "ct": np.ascontiguousarray(c[b].T).astype(np.float16),
            "qt": np.ascontiguousarray(q[b].T).astype(np.float16),
            "wconsts": wconsts,
        }
        for b in range(B)
    ]
    res = run_bass_kernel_spmd(nc, in_maps, core_ids=list(range(B)), trace=TRACE)
    LAST_RESULTS = res
    return np.stack([res.results[b]["s"].astype(np.float32) for b in range(B)],
                    axis=0)
